# revision 15
# baseline (speedup 1.0000x reference)
"""Causal attention with ALiBi for nn_CausalAttention (B=4, T=2048, C=1024,
16 heads) on 8 TRN2 NeuronCores.

Sharding: batch (4) x head-group (2 groups of 8 heads) -> 8 cores.
Each core computes, for its batch b and head group g:
  qT/kT = (Wg.T @ x.T) projections in [d, t] layout, v in [t, d] layout,
  per head: sT[j, i] = qk/8 + slope*(j - i) via an augmented one-hot bias
  matmul (per-column -slope*i, numerically cancels in softmax) plus an ACT
  exp bias of +slope*j (exact fp32); causal masking by gpsimd affine_select
  on the 128x128 triangle block only (also kills Inf from masked overflow);
  PV with an appended ones column gives the softmax denominator;
  normalization via vector.reciprocal + gpsimd partition_broadcast; final
  y_partial = oT.T @ Wo_rows.  Host sums the two head-group partials.

v2: x and all weights are pre-cast to bf16 on the host (device matmuls ran
in bf16 anyway, so results are identical) and DMAd in large batched
transfers; the causal-diagonal scores are trimmed to the valid columns
(QK matmul N = 512-128r, PV reads the valid slice, exp covers a flat
trimmed range with the diagonal tile ordered last in each group); the
normalization chain runs on pooled tiles so successive (head, chunk)
chains pipeline instead of serializing.
"""

import math

import numpy as np

import concourse.bass as bass
import concourse.mybir as mybir
import concourse.tile as tile
from concourse import bacc
from concourse.bass_utils import run_bass_kernel_spmd

B, T, C = 4, 2048, 1024
NH, HD = 16, 64
NHC = 8  # heads per core
BLOCK_SIZE = 2048
NJB = T // 128  # 16 j-blocks
NCH = T // 512  # 4 i-chunks
P = 128

f32 = mybir.dt.float32
bf16 = mybir.dt.bfloat16

LAST_RESULTS = None
_NC_CACHE = None


def get_slopes(n):
    def pow2(n):
        start = 2 ** (-(2 ** (-(math.log2(n) - 3))))
        return [start * start**i for i in range(n)]

    if math.log2(n).is_integer():
        return pow2(n)
    c = 2 ** math.floor(math.log2(n))
    return pow2(c) + get_slopes(2 * c)[0::2][: n - c]


# flat-packed pT column offsets: per jb the active chunks are c in [c0, 4),
# c0=jb//4, stored in order [c0+1, ..., 3, c0] (diagonal chunk LAST, trimmed
# to 512-128r cols, so each exp group is a flat contiguous range ending in
# the trimmed diagonal tile).
_PT_ROWOFF = []
_o = 0
for _jb in range(NJB):
    _PT_ROWOFF.append(_o)
    _c0, _r = _jb // 4, _jb % 4
    _o += (NCH - 1 - _c0) * 512 + (512 - 128 * _r)
NPTC = _o  # 17408 flat columns per head


def _pt_off(jb, c):
    """Column offset of tile (jb, c) in the flat pT slab (and its width)."""
    c0, r = jb // 4, jb % 4
    if c > c0:
        return _PT_ROWOFF[jb] + (c - c0 - 1) * 512, 512
    return _PT_ROWOFF[jb] + (NCH - 1 - c0) * 512, 512 - 128 * r


def build_kernel():
    nc = bacc.Bacc("TRN2", target_bir_lowering=False, debug=False, num_devices=8)

    # x pre-chunked on host: [tck, p, cb, 512] so each per-tck DMA reads
    # 8KB contiguous per partition (full DMA bandwidth)
    xh_d = nc.dram_tensor("xh", [NCH, P, 8, 512], bf16, kind="ExternalInput").ap()
    # wq/wk pre-split per head-pair m: [m, p, cb, 128] (2KB contiguous)
    wqh_d = nc.dram_tensor("wqh", [4, P, 8, P], bf16, kind="ExternalInput").ap()
    wkh_d = nc.dram_tensor("wkh", [4, P, 8, P], bf16, kind="ExternalInput").ap()
    wv_d = nc.dram_tensor("wvb", [C, 512], bf16, kind="ExternalInput").ap()
    wo_d = nc.dram_tensor("wob", [512, C], bf16, kind="ExternalInput").ap()
    qaug_d = nc.dram_tensor("qaugb", [8, NHC, T], bf16, kind="ExternalInput").ap()
    kaug_d = nc.dram_tensor("kaugb", [8, NHC, T], bf16, kind="ExternalInput").ap()
    biasj_d = nc.dram_tensor("biasj", [P, NHC, NJB], f32, kind="ExternalInput").ap()
    y_d = nc.dram_tensor("y", [T, C], f32, kind="ExternalOutput").ap()

    wv_r = wv_d.rearrange("(cb p) m -> p cb m", p=P)  # [128, 8, 512]
    wo_r = wo_d.rearrange("(mb p) n -> p mb n", p=P)  # [128, 4, 1024]
    y_r = y_d.rearrange("(tb p) c -> p tb c", p=P)  # [128, 16, 1024]

    with tile.TileContext(nc) as tc:
        with (
            tc.tile_pool(name="persist", bufs=1) as persist,
            tc.tile_pool(name="work", bufs=2) as work,
            tc.tile_pool(name="psA", bufs=2, space="PSUM") as psA,
            tc.tile_pool(name="psB", bufs=2, space="PSUM") as psB,
            tc.tile_pool(name="psC", bufs=2, space="PSUM") as psC,
        ):
            # ---- persistent tiles ----
            # qT2/kT2: per head h, rows 0-63 = head data (d), rows 64-71 =
            # augmented bias rows; K=72 matmul contracts both at once.
            qT2 = persist.tile([72, NHC, T], bf16)
            kT2 = persist.tile([72, NHC, T], bf16)
            vaug = persist.tile([P, NJB, NHC, 66], bf16)
            oT = persist.tile([P, 4, T], bf16)
            biasj = persist.tile([P, NHC, NJB], f32)

            # ---- fused projections + attention ----
            xp1_cm = tc.tile_pool(name="xp1", bufs=2)
            xp1 = xp1_cm.__enter__()
            wqk_cm = tc.tile_pool(name="wqk", bufs=2)
            wqk = wqk_cm.__enter__()

            def load_x_chunk(tck):
                xts = xp1.tile([P, 8, 512], bf16, tag="xts")
                nc.sync.dma_start(xts[:], xh_d[tck])
                return xts

            # --- v projection ---
            with tc.tile_pool(name="wvp", bufs=1) as wvp:
                # first-consumed DMAs first so V matmuls start ASAP
                wvr = wvp.tile([P, 8, 512], bf16)
                nc.sync.dma_start(wvr[:], wv_r[:])
                xts0 = load_x_chunk(0)
                nc.gpsimd.memset(vaug[:, :, :, 64:66], 1.0)
                nc.scalar.dma_start(biasj[:], biasj_d[:])
                # aug rows: kT2 row 64+r of head h is 1.0 iff r == h;
                # qT2 row 64+r of every head = -slope_r * i
                nc.scalar.dma_start(kT2[64:72, :, :], kaug_d[:])
                nc.scalar.dma_start(qT2[64:72, :, :], qaug_d[:])
                for tck in range(NCH):
                    xts = xts0 if tck == 0 else load_x_chunk(tck)
                    for tb in range(4):
                        psv = psB.tile([P, 512], f32, tag="pb")
                        for c in range(8):
                            nc.tensor.matmul(
                                psv[:],
                                xts[:, c, bass.ts(tb, P)],
                                wvr[:, c, :],
                                start=(c == 0),
                                stop=(c == 7),
                            )
                        nc.vector.tensor_copy(
                            vaug[:, 4 * tck + tb, :, 0:64],
                            psv[:].rearrange("p (h d) -> p h d", h=NHC),
                        )

            def project_pair(m):
                # this pair's weight columns, one batched bf16 DMA each
                wqm = wqk.tile([P, 8, P], bf16, tag="wqm")
                wkm = wqk.tile([P, 8, P], bf16, tag="wkm")
                nc.sync.dma_start(wqm[:], wqh_d[m])
                nc.sync.dma_start(wkm[:], wkh_d[m])
                for tck in range(NCH):
                    xts = load_x_chunk(tck)
                    psq = psB.tile([P, 512], f32, tag="pb")
                    psk = psB.tile([P, 512], f32, tag="pb")
                    for c in range(8):
                        nc.tensor.matmul(
                            psq[:],
                            wqm[:, c, :],
                            xts[:, c, :],
                            start=(c == 0),
                            stop=(c == 7),
                        )
                        nc.tensor.matmul(
                            psk[:],
                            wkm[:, c, :],
                            xts[:, c, :],
                            start=(c == 0),
                            stop=(c == 7),
                        )
                    qstag = work.tile([P, 512], bf16, tag="qkstag")
                    kstag = work.tile([P, 512], bf16, tag="qkstag")
                    nc.vector.tensor_copy(qstag[:], psq[:])
                    nc.vector.tensor_copy(kstag[:], psk[:])
                    tsl = bass.ts(tck, 512)
                    nc.sync.dma_start(qT2[0:64, 2 * m, tsl], qstag[0:64, :])
                    nc.sync.dma_start(qT2[0:64, 2 * m + 1, tsl], qstag[64:128, :])
                    nc.sync.dma_start(kT2[0:64, 2 * m, tsl], kstag[0:64, :])
                    nc.sync.dma_start(kT2[0:64, 2 * m + 1, tsl], kstag[64:128, :])

            ptp_cm = tc.tile_pool(name="ptp", bufs=2)
            ptp = ptp_cm.__enter__()
            pT_of = {}

            def emit_qk(h):
                pT = ptp.tile([P, NPTC], bf16, tag="pT")
                pT_of[h] = pT
                for jb in range(NJB):
                    c0 = jb // 4
                    r = jb % 4
                    dw = 512 - 128 * r  # trimmed diagonal width
                    # tile order within jb: fulls (c0+1..3) then diagonal (c0)
                    order = list(range(c0 + 1, NCH)) + [c0]
                    widths = [512] * (NCH - 1 - c0) + [dw]
                    row0 = _PT_ROWOFF[jb]
                    nact = len(order)
                    for g0 in range(0, nact, 2):
                        ng = min(2, nact - g0)
                        gw = sum(widths[g0 : g0 + ng])
                        ssum = psA.tile([P, 2, 512], f32, tag="ssum")
                        for ci in range(ng):
                            c = order[g0 + ci]
                            w = widths[g0 + ci]
                            if c == c0 and r > 0:
                                # diagonal: compute only cols i_off>=128r,
                                # stored left-aligned in its slot
                                rhs = qT2[:, h, 512 * c + 128 * r : 512 * (c + 1)]
                            else:
                                rhs = qT2[:, h, bass.ts(c, 512)]
                            nc.tensor.matmul(
                                ssum[:, ci, 0:w],
                                kT2[:, h, bass.ts(jb, P)],
                                rhs,
                                start=True,
                                stop=True,
                            )
                        sflat = ssum[:].rearrange("p a b -> p (a b)")
                        goff = row0 + g0 * 512
                        nc.scalar.activation(
                            pT[:, goff : goff + gw],
                            sflat[:, 0:gw],
                            mybir.ActivationFunctionType.Exp,
                            bias=biasj[:, h, jb : jb + 1],
                            scale=1.0,
                        )
                    # causal mask on the 128-wide triangle block of the
                    # diagonal tile (left-aligned at its slot after the trim):
                    # keep where f' - p >= 0
                    doff, _ = _pt_off(jb, c0)
                    nc.gpsimd.affine_select(
                        pT[:, doff : doff + 128],
                        pT[:, doff : doff + 128],
                        pattern=[[1, 128]],
                        compare_op=mybir.AluOpType.is_ge,
                        fill=0.0,
                        base=0,
                        channel_multiplier=-1,
                    )

            norm_cm = tc.tile_pool(name="norm", bufs=3)
            norm = norm_cm.__enter__()
            normb_cm = tc.tile_pool(name="normb", bufs=2)
            normb = normb_cm.__enter__()

            def emit_pv(h):
                hp = (h % 2) * 64
                hm = h // 2
                pT = pT_of.pop(h)
                for c in range(NCH):
                    pot = psC.tile([65, 512], f32, tag="pot")
                    njb = 4 * c + 4
                    for jb in range(njb):
                        off, w = _pt_off(jb, c)
                        if w < 512:
                            # diagonal block: valid cols start at 512-w
                            nc.tensor.matmul(
                                pot[:, 512 - w : 512],
                                vaug[:, jb, h, 0:65],
                                pT[:, off : off + w],
                                start=False,
                                stop=(jb == njb - 1),
                            )
                        else:
                            nc.tensor.matmul(
                                pot[:],
                                vaug[:, jb, h, 0:65],
                                pT[:, off : off + w],
                                start=(jb == 0),
                                stop=(jb == njb - 1),
                            )
                    # copy out fast to release the PSUM bank, then normalize
                    # off the PV critical path (pooled tiles so successive
                    # chains pipeline).
                    potsb = norm.tile([65, 512], f32, tag="potsb")
                    nc.vector.tensor_copy(potsb[:], pot[:])
                    # spread the 512 rowsums across 128 partitions so the
                    # reciprocal uses all DVE lanes (26ns vs 3.3us)
                    rs128 = normb.tile([P, 4], f32, tag="rs128")
                    nc.scalar.dma_start(rs128[:], potsb[64:65, :])
                    nc.vector.reciprocal(rs128[:], rs128[:])
                    srecip = normb.tile([1, 512], f32, tag="srecip")
                    nc.scalar.dma_start(srecip[:], rs128[:])
                    bcast = normb.tile([64, 512], f32, tag="bcast")
                    nc.gpsimd.partition_broadcast(bcast[:], srecip[:])
                    nc.vector.tensor_tensor(
                        oT[hp : hp + 64, hm, bass.ts(c, 512)],
                        potsb[0:64, :],
                        bcast[:],
                        mybir.AluOpType.mult,
                    )

            for m in range(4):
                project_pair(m)
                emit_qk(2 * m)
                if m > 0:
                    emit_pv(2 * m - 1)
                emit_qk(2 * m + 1)
                emit_pv(2 * m)
            emit_pv(NHC - 1)

            normb_cm.__exit__(None, None, None)
            norm_cm.__exit__(None, None, None)
            ptp_cm.__exit__(None, None, None)
            wqk_cm.__exit__(None, None, None)
            xp1_cm.__exit__(None, None, None)

            # ---- output projection ----
            with (
                tc.tile_pool(name="wop", bufs=1) as wop,
                tc.tile_pool(name="ypool", bufs=3) as ypool,
            ):
                wor = wop.tile([P, 4, C], bf16)
                nc.sync.dma_start(wor[:], wo_r[:])

                for tb in range(NJB):
                    for cc in range(2):
                        psy = psB.tile([P, 512], f32, tag="pb")
                        for m in range(4):
                            nc.tensor.matmul(
                                psy[:],
                                oT[:, m, bass.ts(tb, P)],
                                wor[:, m, bass.ts(cc, 512)],
                                start=(m == 0),
                                stop=(m == 3),
                            )
                        ysb = ypool.tile([P, 512], f32, tag="ysb")
                        nc.vector.tensor_copy(ysb[:], psy[:])
                        nc.sync.dma_start(y_r[:, tb, bass.ts(cc, 512)], ysb[:])

    nc.compile()
    return nc


def kernel(x, Wq, Wk, Wv, Wo):
    global LAST_RESULTS, _NC_CACHE
    import ml_dtypes

    x = np.asarray(x, dtype=np.float32)
    Wq = np.asarray(Wq, dtype=np.float32)
    Wk = np.asarray(Wk, dtype=np.float32)
    Wv = np.asarray(Wv, dtype=np.float32)
    Wo = np.asarray(Wo, dtype=np.float32)

    slopes = np.asarray(get_slopes(NH), dtype=np.float32)
    ii = np.arange(T, dtype=np.float64)
    pp = np.arange(P, dtype=np.float64)

    if _NC_CACHE is None:
        _NC_CACHE = build_kernel()
    nc = _NC_CACHE

    in_maps = []
    for core in range(8):
        b, g = core // 2, core % 2
        hsl = slice(g * 512, (g + 1) * 512)
        core_slopes = slopes[g * NHC : (g + 1) * NHC].astype(np.float64)

        qaug1 = (-core_slopes[:, None] * ii[None, :]).astype(ml_dtypes.bfloat16)
        qaugb = np.ascontiguousarray(
            np.broadcast_to(qaug1[:, None, :], (8, NHC, T))
        )
        kaugb = np.zeros((8, NHC, T), ml_dtypes.bfloat16)
        for h in range(NHC):
            kaugb[h, h, :] = ml_dtypes.bfloat16(1.0)
        biasj = np.zeros((P, NHC, NJB), np.float32)
        for h in range(NHC):
            for jb in range(NJB):
                biasj[:, h, jb] = (core_slopes[h] * (128 * jb + pp)).astype(np.float32)
        # xh[tck, p, cb, t'] = x[b].T[cb*128+p, tck*512+t']
        xT = x[b].T.astype(ml_dtypes.bfloat16)  # [C, T]
        xh = np.ascontiguousarray(
            xT.reshape(8, P, NCH, 512).transpose(2, 1, 0, 3)
        )
        # wqh[m, p, cb, mm] = (0.125*Wq)[cb*128+p, m*128+mm] on this head group
        wqs = (Wq[:, hsl] * np.float32(0.125)).astype(ml_dtypes.bfloat16)
        wks = Wk[:, hsl].astype(ml_dtypes.bfloat16)
        wqh = np.ascontiguousarray(
            wqs.reshape(8, P, 4, P).transpose(2, 1, 0, 3)
        )
        wkh = np.ascontiguousarray(
            wks.reshape(8, P, 4, P).transpose(2, 1, 0, 3)
        )
        in_maps.append(
            {
                "xh": xh,
                "wqh": wqh,
                "wkh": wkh,
                "wvb": np.ascontiguousarray(Wv[:, hsl]).astype(ml_dtypes.bfloat16),
                "wob": np.ascontiguousarray(Wo[hsl, :]).astype(ml_dtypes.bfloat16),
                "qaugb": qaugb,
                "kaugb": kaugb,
                "biasj": biasj,
            }
        )

    res = run_bass_kernel_spmd(nc, in_maps, list(range(8)))
    LAST_RESULTS = res
    out = np.empty((B, T, C), dtype=np.float32)
    for b in range(B):
        out[b] = res.results[2 * b]["y"] + res.results[2 * b + 1]["y"]
    return out


# revision 20
# speedup vs baseline: 1.1796x; 1.1796x over previous
"""Causal attention with ALiBi for nn_CausalAttention (B=4, T=2048, C=1024,
16 heads) on 8 TRN2 NeuronCores.

Sharding: batch (4) x head-group (2 groups of 8 heads) -> 8 cores.
Each core computes, for its batch b and head group g:
  qT/kT = (Wg.T @ x.T) projections in [d, t] layout, v in [t, d] layout,
  per head: sT[j, i] = qk/8 + slope*(j - i) via an augmented one-hot bias
  matmul (per-column -slope*i, numerically cancels in softmax) plus an ACT
  exp bias of +slope*j (exact fp32); causal masking by gpsimd affine_select
  on the 128x128 triangle block only (also kills Inf from masked overflow);
  PV with an appended ones column gives the softmax denominator;
  normalization via vector.reciprocal + gpsimd partition_broadcast; final
  y_partial = oT.T @ Wo_rows.  Host sums the two head-group partials.

v2: x and all weights are pre-cast to bf16 on the host (device matmuls ran
in bf16 anyway, so results are identical) and DMAd in large batched
transfers; the causal-diagonal scores are trimmed to the valid columns
(QK matmul N = 512-128r, PV reads the valid slice, exp covers a flat
trimmed range with the diagonal tile ordered last in each group); the
normalization chain runs on pooled tiles so successive (head, chunk)
chains pipeline instead of serializing.
"""

import math

import numpy as np

import concourse.bass as bass
import concourse.mybir as mybir
import concourse.tile as tile
from concourse import bacc
from concourse.bass_utils import run_bass_kernel_spmd

B, T, C = 4, 2048, 1024
NH, HD = 16, 64
NHC = 8  # heads per core
BLOCK_SIZE = 2048
NJB = T // 128  # 16 j-blocks
NCH = T // 512  # 4 i-chunks
P = 128

f32 = mybir.dt.float32
bf16 = mybir.dt.bfloat16

LAST_RESULTS = None
_NC_CACHE = None


def get_slopes(n):
    def pow2(n):
        start = 2 ** (-(2 ** (-(math.log2(n) - 3))))
        return [start * start**i for i in range(n)]

    if math.log2(n).is_integer():
        return pow2(n)
    c = 2 ** math.floor(math.log2(n))
    return pow2(c) + get_slopes(2 * c)[0::2][: n - c]


# flat-packed pT column offsets: per jb the active chunks are c in [c0, 4),
# c0=jb//4, stored in order [c0+1, ..., 3, c0] (diagonal chunk LAST, trimmed
# to 512-128r cols, so each exp group is a flat contiguous range ending in
# the trimmed diagonal tile).
_PT_ROWOFF = []
_o = 0
for _jb in range(NJB):
    _PT_ROWOFF.append(_o)
    _c0, _r = _jb // 4, _jb % 4
    _o += (NCH - 1 - _c0) * 512 + (512 - 128 * _r)
NPTC = _o  # 17408 flat columns per head


def _pt_off(jb, c):
    """Column offset of tile (jb, c) in the flat pT slab (and its width)."""
    c0, r = jb // 4, jb % 4
    if c > c0:
        return _PT_ROWOFF[jb] + (c - c0 - 1) * 512, 512
    return _PT_ROWOFF[jb] + (NCH - 1 - c0) * 512, 512 - 128 * r


def build_kernel():
    nc = bacc.Bacc("TRN2", target_bir_lowering=False, debug=False, num_devices=8)

    # x pre-chunked on host: [tck, p, cb, 512] so each per-tck DMA reads
    # 8KB contiguous per partition (full DMA bandwidth)
    xh_d = nc.dram_tensor("xh", [NCH, P, 8, 512], bf16, kind="ExternalInput").ap()
    # wq/wk pre-split per head-pair m: [m, p, cb, 128] (2KB contiguous)
    wqh_d = nc.dram_tensor("wqh", [4, P, 8, P], bf16, kind="ExternalInput").ap()
    wkh_d = nc.dram_tensor("wkh", [4, P, 8, P], bf16, kind="ExternalInput").ap()
    # wv pre-arranged: [p, cb, 512] so the load is 8KB contiguous/partition
    wvh_d = nc.dram_tensor("wvh", [P, 8, 512], bf16, kind="ExternalInput").ap()
    wo_d = nc.dram_tensor("wob", [512, C], bf16, kind="ExternalInput").ap()
    qaug_d = nc.dram_tensor("qaugb", [8, NHC, T], bf16, kind="ExternalInput").ap()
    kaug_d = nc.dram_tensor("kaugb", [8, NHC, T], bf16, kind="ExternalInput").ap()
    biasj_d = nc.dram_tensor("biasj", [P, NHC, NJB], f32, kind="ExternalInput").ap()
    y_d = nc.dram_tensor("y", [T, C], f32, kind="ExternalOutput").ap()

    wo_r = wo_d.rearrange("(mb p) n -> p mb n", p=P)  # [128, 4, 1024]
    y_r = y_d.rearrange("(tb p) c -> p tb c", p=P)  # [128, 16, 1024]

    with tile.TileContext(nc) as tc:
        with (
            tc.tile_pool(name="persist", bufs=1) as persist,
            tc.tile_pool(name="work", bufs=2) as work,
            tc.tile_pool(name="psA", bufs=2, space="PSUM") as psA,
            tc.tile_pool(name="psB", bufs=2, space="PSUM") as psB,
            tc.tile_pool(name="psC", bufs=2, space="PSUM") as psC,
        ):
            # ---- persistent tiles ----
            # qT2/kT2: per head h, rows 0-63 = head data (d), rows 64-71 =
            # augmented bias rows; K=72 matmul contracts both at once.
            qT2 = persist.tile([72, NHC, T], bf16)
            kT2 = persist.tile([72, NHC, T], bf16)
            vaug = persist.tile([P, NJB, NHC, 66], bf16)
            oT = persist.tile([P, 4, T], bf16)
            biasj = persist.tile([P, NHC, NJB], f32)

            # ---- fused projections + attention ----
            xp1_cm = tc.tile_pool(name="xp1", bufs=2)
            xp1 = xp1_cm.__enter__()
            wqk_cm = tc.tile_pool(name="wqk", bufs=2)
            wqk = wqk_cm.__enter__()

            def load_x_chunk(tck):
                xts = xp1.tile([P, 8, 512], bf16, tag="xts")
                nc.sync.dma_start(xts[:], xh_d[tck])
                return xts

            # --- v projection ---
            with tc.tile_pool(name="wvp", bufs=1) as wvp:
                # first-consumed DMAs first so V matmuls start ASAP
                wvr = wvp.tile([P, 8, 512], bf16)
                nc.sync.dma_start(wvr[:], wvh_d[:])
                xts0 = load_x_chunk(0)
                nc.gpsimd.memset(vaug[:, :, :, 64:66], 1.0)
                nc.scalar.dma_start(biasj[:], biasj_d[:])
                # aug rows: kT2 row 64+r of head h is 1.0 iff r == h;
                # qT2 row 64+r of every head = -slope_r * i
                nc.scalar.dma_start(kT2[64:72, :, :], kaug_d[:])
                nc.scalar.dma_start(qT2[64:72, :, :], qaug_d[:])
                for tck in range(NCH):
                    xts = xts0 if tck == 0 else load_x_chunk(tck)
                    for tb in range(4):
                        psv = psB.tile([P, 512], f32, tag="pb")
                        for c in range(8):
                            nc.tensor.matmul(
                                psv[:],
                                xts[:, c, bass.ts(tb, P)],
                                wvr[:, c, :],
                                start=(c == 0),
                                stop=(c == 7),
                            )
                        nc.vector.tensor_copy(
                            vaug[:, 4 * tck + tb, :, 0:64],
                            psv[:].rearrange("p (h d) -> p h d", h=NHC),
                        )

            def project_pair(m):
                # this pair's weight columns, one batched bf16 DMA each
                wqm = wqk.tile([P, 8, P], bf16, tag="wqm")
                wkm = wqk.tile([P, 8, P], bf16, tag="wkm")
                nc.sync.dma_start(wqm[:], wqh_d[m])
                nc.sync.dma_start(wkm[:], wkh_d[m])
                for tck in range(NCH):
                    xts = load_x_chunk(tck)
                    psq = psB.tile([P, 512], f32, tag="pb")
                    psk = psB.tile([P, 512], f32, tag="pb")
                    for c in range(8):
                        nc.tensor.matmul(
                            psq[:],
                            wqm[:, c, :],
                            xts[:, c, :],
                            start=(c == 0),
                            stop=(c == 7),
                        )
                        nc.tensor.matmul(
                            psk[:],
                            wkm[:, c, :],
                            xts[:, c, :],
                            start=(c == 0),
                            stop=(c == 7),
                        )
                    qstag = work.tile([P, 512], bf16, tag="qkstag")
                    kstag = work.tile([P, 512], bf16, tag="qkstag")
                    nc.vector.tensor_copy(qstag[:], psq[:])
                    nc.vector.tensor_copy(kstag[:], psk[:])
                    tsl = bass.ts(tck, 512)
                    nc.sync.dma_start(qT2[0:64, 2 * m, tsl], qstag[0:64, :])
                    nc.sync.dma_start(qT2[0:64, 2 * m + 1, tsl], qstag[64:128, :])
                    nc.sync.dma_start(kT2[0:64, 2 * m, tsl], kstag[0:64, :])
                    nc.sync.dma_start(kT2[0:64, 2 * m + 1, tsl], kstag[64:128, :])

            ptp_cm = tc.tile_pool(name="ptp", bufs=2)
            ptp = ptp_cm.__enter__()
            pT_of = {}

            def emit_qk(h):
                pT = ptp.tile([P, NPTC], bf16, tag="pT")
                pT_of[h] = pT
                for jb in range(NJB):
                    c0 = jb // 4
                    r = jb % 4
                    dw = 512 - 128 * r  # trimmed diagonal width
                    # tile order within jb: fulls (c0+1..3) then diagonal (c0)
                    order = list(range(c0 + 1, NCH)) + [c0]
                    widths = [512] * (NCH - 1 - c0) + [dw]
                    row0 = _PT_ROWOFF[jb]
                    nact = len(order)
                    for g0 in range(0, nact, 2):
                        ng = min(2, nact - g0)
                        gw = sum(widths[g0 : g0 + ng])
                        ssum = psA.tile([P, 2, 512], f32, tag="ssum")
                        for ci in range(ng):
                            c = order[g0 + ci]
                            w = widths[g0 + ci]
                            if c == c0 and r > 0:
                                # diagonal: compute only cols i_off>=128r,
                                # stored left-aligned in its slot
                                rhs = qT2[:, h, 512 * c + 128 * r : 512 * (c + 1)]
                            else:
                                rhs = qT2[:, h, bass.ts(c, 512)]
                            nc.tensor.matmul(
                                ssum[:, ci, 0:w],
                                kT2[:, h, bass.ts(jb, P)],
                                rhs,
                                start=True,
                                stop=True,
                            )
                        sflat = ssum[:].rearrange("p a b -> p (a b)")
                        goff = row0 + g0 * 512
                        nc.scalar.activation(
                            pT[:, goff : goff + gw],
                            sflat[:, 0:gw],
                            mybir.ActivationFunctionType.Exp,
                            bias=biasj[:, h, jb : jb + 1],
                            scale=1.0,
                        )
                    # causal mask on the 128-wide triangle block of the
                    # diagonal tile (left-aligned at its slot after the trim):
                    # keep where f' - p >= 0
                    doff, _ = _pt_off(jb, c0)
                    nc.gpsimd.affine_select(
                        pT[:, doff : doff + 128],
                        pT[:, doff : doff + 128],
                        pattern=[[1, 128]],
                        compare_op=mybir.AluOpType.is_ge,
                        fill=0.0,
                        base=0,
                        channel_multiplier=-1,
                    )

            norm_cm = tc.tile_pool(name="norm", bufs=3)
            norm = norm_cm.__enter__()
            normb_cm = tc.tile_pool(name="normb", bufs=2)
            normb = normb_cm.__enter__()

            def emit_pv(h):
                hp = (h % 2) * 64
                hm = h // 2
                pT = pT_of.pop(h)
                for c in range(NCH):
                    pot = psC.tile([65, 512], f32, tag="pot")
                    njb = 4 * c + 4
                    for jb in range(njb):
                        off, w = _pt_off(jb, c)
                        if w < 512:
                            # diagonal block: valid cols start at 512-w
                            nc.tensor.matmul(
                                pot[:, 512 - w : 512],
                                vaug[:, jb, h, 0:65],
                                pT[:, off : off + w],
                                start=False,
                                stop=(jb == njb - 1),
                            )
                        else:
                            nc.tensor.matmul(
                                pot[:],
                                vaug[:, jb, h, 0:65],
                                pT[:, off : off + w],
                                start=(jb == 0),
                                stop=(jb == njb - 1),
                            )
                    # copy out fast to release the PSUM bank, then normalize
                    # off the PV critical path (pooled tiles so successive
                    # chains pipeline).
                    potsb = norm.tile([65, 512], f32, tag="potsb")
                    nc.vector.tensor_copy(potsb[:], pot[:])
                    # spread the 512 rowsums across 128 partitions so the
                    # reciprocal uses all DVE lanes (26ns vs 3.3us)
                    rs128 = normb.tile([P, 4], f32, tag="rs128")
                    nc.sync.dma_start(rs128[:], potsb[64:65, :])
                    nc.vector.reciprocal(rs128[:], rs128[:])
                    srecip = normb.tile([1, 512], f32, tag="srecip")
                    nc.sync.dma_start(srecip[:], rs128[:])
                    bcast = normb.tile([64, 512], f32, tag="bcast")
                    nc.gpsimd.partition_broadcast(bcast[:], srecip[:])
                    nc.vector.tensor_tensor(
                        oT[hp : hp + 64, hm, bass.ts(c, 512)],
                        potsb[0:64, :],
                        bcast[:],
                        mybir.AluOpType.mult,
                    )

            for m in range(4):
                project_pair(m)
                emit_qk(2 * m)
                if m > 0:
                    emit_pv(2 * m - 1)
                emit_qk(2 * m + 1)
                emit_pv(2 * m)
            emit_pv(NHC - 1)

            normb_cm.__exit__(None, None, None)
            norm_cm.__exit__(None, None, None)
            ptp_cm.__exit__(None, None, None)
            wqk_cm.__exit__(None, None, None)
            xp1_cm.__exit__(None, None, None)

            # ---- output projection ----
            with (
                tc.tile_pool(name="wop", bufs=1) as wop,
                tc.tile_pool(name="ypool", bufs=3) as ypool,
            ):
                wor = wop.tile([P, 4, C], bf16)
                nc.sync.dma_start(wor[:], wo_r[:])

                for tb in range(NJB):
                    for cc in range(2):
                        psy = psB.tile([P, 512], f32, tag="pb")
                        for m in range(4):
                            nc.tensor.matmul(
                                psy[:],
                                oT[:, m, bass.ts(tb, P)],
                                wor[:, m, bass.ts(cc, 512)],
                                start=(m == 0),
                                stop=(m == 3),
                            )
                        ysb = ypool.tile([P, 512], f32, tag="ysb")
                        nc.vector.tensor_copy(ysb[:], psy[:])
                        nc.sync.dma_start(y_r[:, tb, bass.ts(cc, 512)], ysb[:])

    nc.compile()
    return nc


def kernel(x, Wq, Wk, Wv, Wo):
    global LAST_RESULTS, _NC_CACHE
    import ml_dtypes

    x = np.asarray(x, dtype=np.float32)
    Wq = np.asarray(Wq, dtype=np.float32)
    Wk = np.asarray(Wk, dtype=np.float32)
    Wv = np.asarray(Wv, dtype=np.float32)
    Wo = np.asarray(Wo, dtype=np.float32)

    slopes = np.asarray(get_slopes(NH), dtype=np.float32)
    ii = np.arange(T, dtype=np.float64)
    pp = np.arange(P, dtype=np.float64)

    if _NC_CACHE is None:
        _NC_CACHE = build_kernel()
    nc = _NC_CACHE

    in_maps = []
    for core in range(8):
        b, g = core // 2, core % 2
        hsl = slice(g * 512, (g + 1) * 512)
        core_slopes = slopes[g * NHC : (g + 1) * NHC].astype(np.float64)

        qaug1 = (-core_slopes[:, None] * ii[None, :]).astype(ml_dtypes.bfloat16)
        qaugb = np.ascontiguousarray(
            np.broadcast_to(qaug1[:, None, :], (8, NHC, T))
        )
        kaugb = np.zeros((8, NHC, T), ml_dtypes.bfloat16)
        for h in range(NHC):
            kaugb[h, h, :] = ml_dtypes.bfloat16(1.0)
        biasj = np.zeros((P, NHC, NJB), np.float32)
        for h in range(NHC):
            for jb in range(NJB):
                biasj[:, h, jb] = (core_slopes[h] * (128 * jb + pp)).astype(np.float32)
        # xh[tck, p, cb, t'] = x[b].T[cb*128+p, tck*512+t']
        xT = x[b].T.astype(ml_dtypes.bfloat16)  # [C, T]
        xh = np.ascontiguousarray(
            xT.reshape(8, P, NCH, 512).transpose(2, 1, 0, 3)
        )
        # wqh[m, p, cb, mm] = (0.125*Wq)[cb*128+p, m*128+mm] on this head group
        wqs = (Wq[:, hsl] * np.float32(0.125)).astype(ml_dtypes.bfloat16)
        wks = Wk[:, hsl].astype(ml_dtypes.bfloat16)
        wqh = np.ascontiguousarray(
            wqs.reshape(8, P, 4, P).transpose(2, 1, 0, 3)
        )
        wkh = np.ascontiguousarray(
            wks.reshape(8, P, 4, P).transpose(2, 1, 0, 3)
        )
        in_maps.append(
            {
                "xh": xh,
                "wqh": wqh,
                "wkh": wkh,
                "wvh": np.ascontiguousarray(
                    Wv[:, hsl]
                    .astype(ml_dtypes.bfloat16)
                    .reshape(8, P, 512)
                    .transpose(1, 0, 2)
                ),
                "wob": np.ascontiguousarray(Wo[hsl, :]).astype(ml_dtypes.bfloat16),
                "qaugb": qaugb,
                "kaugb": kaugb,
                "biasj": biasj,
            }
        )

    res = run_bass_kernel_spmd(nc, in_maps, list(range(8)))
    LAST_RESULTS = res
    out = np.empty((B, T, C), dtype=np.float32)
    for b in range(B):
        out[b] = res.results[2 * b]["y"] + res.results[2 * b + 1]["y"]
    return out
